# revision 72
# baseline (speedup 1.0000x reference)
"""BatchGAT Trainium2 kernel (Bass/Tile), data-parallel over the 8 subgraphs.

Per core (1 subgraph, n=1024 nodes, 8 heads, 2 GAT layers), the attention
matrix exp(leakyrelu(s_n + d_m)) is never exponentiated elementwise.
Using softmax's invariance to per-column (per-destination-node) scales:

  exp(lrelu(s+d)) = E2S[n] * max(exp(d_m)*exp(0.8 s_n), exp(0.2 d_m))

The E2S[n] column factor cancels between numerator and denominator, and
exp(d_m) is a per-partition (source node) scale folded into the matmul
lhsT (h_aug * exp(d)) during its PSUM evacuation.  What remains per
128x1024 attention chunk is ONE DVE op in the 4x perf mode:

  u = (E8S max exp(-0.8 d)_col) * adjT_chunk        (all bf16, SBUF)

followed by the bf16 numerator matmul (ones column scaled by exp(d)
yields the softmax denominator Z).  adj ships from the host as bf16 and
is transposed by the DMA crossbar (dma_start_transpose) straight into
SBUF.  x ships host-transposed; the embedding half of x0T is gathered
by indirect DMA and PE-transposed.  Normalization 1/Z is built in
column form (tiny PE gathers), reciprocal'd in one batched DVE op, and
broadcast back to rows via stride-0-lhsT matmuls against the bf16
identity; layer outputs (elu / head-mean) write their final stacked
layout directly as partition-sliced DVE/GPSIMD stores, so both layers
feed the next matmuls without restacking passes.
"""

import numpy as np

BS, N, VOCAB, EMB, FEAT = 8, 1024, 100000, 64, 64
P = 128
NCH = N // P  # 8 node chunks
H = 8
HALF = 512

# wpack (f32) columns
C_IDENT = 0            # [128,128] identity f32 (PE transpose helper)
C_B0 = C_IDENT + P     # 1 col, partitions 0..31
C_B1 = C_B0 + 1        # 1 col, partitions 0..15
WCOLS = C_B1 + 1

# wpackr (f32r) columns
C_W0 = 0               # 8 heads x 32 cols: w0[h] [128,32]
C_W1 = C_W0 + 8 * 32   # 8 heads x 2 kch x 16 cols: w1 blocks [128,16]
C_AS0 = C_W1 + 8 * 32  # 8 cols a_src0 (rows 0:32)
C_AS1 = C_AS0 + 8      # 8 cols a_src1 (rows 0:16)
C_AD0 = C_AS1 + 8      # 8 heads x 2 cols [a_dst0, -0.8*a_dst0] (rows 0:32)
C_AD1 = C_AD0 + 16     # 8 heads x 2 cols [a_dst1, -0.8*a_dst1] (rows 0:16)
RCOLS = C_AD1 + 16

# wpackb (bf16) columns
CB_ID = 0              # [128,128] identity bf16
CB_OR = CB_ID + P      # ones row: partition 0, 128 cols of 1.0
CB_ONE = CB_OR + P     # 1 col: 1.0 (partition 0)
CB_EIGHT = CB_ONE + 1  # 1 col: 8.0 (partition 0)
CB_B1R = CB_EIGHT + 1  # [128,16]: b1 broadcast to all partitions
CB_O17 = CB_B1R + 16   # 17 cols of 1.0 on partition 0
CB_W1 = CB_O17 + 17    # 8 heads x 2 kch x 16 cols: w1 blocks, bf16
BCOLS = CB_W1 + 8 * 2 * 16

_CACHE = {}


def _build(zero_b0, zero_b1):
    import concourse.bass as bass
    import concourse.tile as tile
    from concourse import bacc, mybir
    from contextlib import ExitStack

    dt = mybir.dt
    f32 = dt.float32
    f32r = dt.float32r
    bf16 = dt.bfloat16
    A = mybir.ActivationFunctionType
    OP = mybir.AluOpType

    nc = bacc.Bacc("TRN2", target_bir_lowering=False, debug=False,
                   dynamic_dma_scratch_size=65536)

    xt_d = nc.dram_tensor("xt", [FEAT, N], f32r, kind="ExternalInput")
    v_d = nc.dram_tensor("verts", [P, NCH], dt.int32, kind="ExternalInput")
    adjb_d = nc.dram_tensor("adjb", [N, N], bf16, kind="ExternalInput")
    emb_d = nc.dram_tensor("emb_w", [VOCAB, EMB], f32, kind="ExternalInput")
    wp_d = nc.dram_tensor("wpack", [P, WCOLS], f32, kind="ExternalInput")
    wpr_d = nc.dram_tensor("wpackr", [P, RCOLS], f32r, kind="ExternalInput")
    wpb_d = nc.dram_tensor("wpackb", [P, BCOLS], bf16, kind="ExternalInput")
    out_d = nc.dram_tensor("out", [N, 16], f32, kind="ExternalOutput")

    with tile.TileContext(nc) as tc, ExitStack() as ctx:
        singles = ctx.enter_context(tc.tile_pool(name="singles", bufs=1))
        eep = ctx.enter_context(tc.tile_pool(name="eep", bufs=8))
        hpool = ctx.enter_context(tc.tile_pool(name="hpool", bufs=2))
        epool = ctx.enter_context(tc.tile_pool(name="epool", bufs=2))
        ddpool = ctx.enter_context(tc.tile_pool(name="ddpool", bufs=4))
        hapool = ctx.enter_context(tc.tile_pool(name="hapool", bufs=6))
        upool = ctx.enter_context(tc.tile_pool(name="upool", bufs=6))
        oupool = ctx.enter_context(tc.tile_pool(name="oupool", bufs=8))
        ypool = ctx.enter_context(tc.tile_pool(name="ypool", bufs=2))
        mpool = ctx.enter_context(tc.tile_pool(name="mpool", bufs=2))
        stg = ctx.enter_context(tc.tile_pool(name="stg", bufs=3))
        pbig = ctx.enter_context(tc.tile_pool(name="pbig", bufs=2, space="PSUM"))
        pattn = ctx.enter_context(tc.tile_pool(name="pattn", bufs=1, space="PSUM"))
        psmall = ctx.enter_context(tc.tile_pool(name="psmall", bufs=2, space="PSUM"))

        # ---- inputs; order the critical x0T path before the adj transposes
        vts = singles.tile([P, NCH], dt.int32, tag="vts")
        nc.sync.dma_start(out=vts[:], in_=v_d[:, :])
        x0T = singles.tile([P, N], f32r, tag="x0T")
        nc.sync.dma_start(out=x0T[0:FEAT, :], in_=xt_d[:, :])
        wpr = singles.tile([P, RCOLS], f32r, tag="wpr")
        nc.sync.dma_start(out=wpr[:], in_=wpr_d[:, :])
        wpb = singles.tile([P, BCOLS], bf16, tag="wpb")
        nc.sync.dma_start(out=wpb[:], in_=wpb_d[:, :])
        identb = wpb[:, CB_ID:CB_ID + P]
        wp = singles.tile([P, WCOLS], f32, tag="wp")
        nc.sync.dma_start(out=wp[:], in_=wp_d[:, :])

        # ---- adjT via DMA crossbar transpose: adjT[p, jc*N+n] = adj[n, jc*128+p]
        adjT = singles.tile([P, NCH * N], bf16, tag="adjT")
        for jc in range(NCH):
            nc.sync.dma_start_transpose(
                out=adjT[:, jc * N:(jc + 1) * N],
                in_=adjb_d[:, jc * P:(jc + 1) * P],
            )

        # PE warmup
        pwarm = psmall.tile([16, 16], f32, tag="sm")
        nc.tensor.matmul(out=pwarm[:], lhsT=wpr[:, 0:16], rhs=wpr[:, 0:16],
                         start=True, stop=True)
        pwarm2 = psmall.tile([16, 16], f32, tag="sm", name="pwarm2")
        nc.tensor.matmul(out=pwarm2[:], lhsT=wpr[:, 0:16], rhs=wpr[:, 0:16],
                         start=True, stop=True)

        for c in range(NCH):
            ee = eep.tile([P, EMB], f32, tag="ee")
            nc.gpsimd.indirect_dma_start(
                out=ee[:],
                out_offset=None,
                in_=emb_d[:, :],
                in_offset=bass.IndirectOffsetOnAxis(ap=vts[:, c:c + 1], axis=0),
            )
            pe_t = psmall.tile([EMB, P], f32, tag="sm", name=f"pet{c}")
            nc.tensor.transpose(
                out=pe_t[:], in_=ee[:], identity=wp[:, C_IDENT:C_IDENT + P]
            )
            nc.vector.tensor_copy(
                out=x0T[FEAT:P, c * P:(c + 1) * P], in_=pe_t[:]
            )

        # ---- layers ----
        x1T = [
            singles.tile([P, N], bf16, tag=f"x1T{k}", name=f"x1T{k}")
            for k in range(2)
        ]
        xT_in = [x0T]
        fon = 33  # attn lhsT rows: fo outputs (+pad) + Z col at partition 32
        for li in range(2):
            fo = 32 if li == 0 else 16
            kch = 1 if li == 0 else 2
            c_w = C_W0 if li == 0 else C_W1
            wblk = 32 if li == 0 else 16
            c_as = C_AS0 if li == 0 else C_AS1
            c_ad = C_AD0 if li == 0 else C_AD1
            zc_rhs = wpb[32:33, CB_ONE:CB_ONE + 1] if li == 0 else \
                wpb[32:33, CB_EIGHT:CB_EIGHT + 1]
            ou_list = []
            y_list = []

            def head_prologue(h):
                # feature-major h' and tanh
                # weight blocks: L0 f32r (pairs with x0T f32r), L1 bf16
                # (pairs with x1T bf16; f32r may not mix with bf16)
                if li == 0:
                    wblks = [wpr[:, c_w + h * 32:c_w + h * 32 + fo]]
                else:
                    wblks = [
                        wpb[:, CB_W1 + (h * 2 + k) * 16:
                            CB_W1 + (h * 2 + k) * 16 + fo]
                        for k in range(2)
                    ]
                ph = pbig.tile([fo, N], f32, tag="big")
                for hf in range(2):
                    fs = slice(hf * HALF, (hf + 1) * HALF)
                    for k in range(kch):
                        nc.tensor.matmul(
                            out=ph[:, fs],
                            lhsT=wblks[k],
                            rhs=xT_in[k][:, fs],
                            start=(k == 0),
                            stop=(k == kch - 1),
                        )
                tT = hpool.tile([fo, N], f32r, tag="tT")
                nc.scalar.activation(out=tT[:], in_=ph[:], func=A.Tanh)

                # s broadcast -> E8S = exp(0.8 s) on all partitions (bf16)
                psb = pbig.tile([P, N], f32, tag="big", name="psb")
                for hf in range(2):
                    fs = slice(hf * HALF, (hf + 1) * HALF)
                    nc.tensor.matmul(
                        out=psb[:, fs],
                        lhsT=wpr[0:fo, c_as + h:c_as + h + 1].to_broadcast(
                            [fo, P]),
                        rhs=tT[:, fs],
                        start=True, stop=True,
                    )
                e8s = epool.tile([P, N], bf16, tag="e8s")
                nc.scalar.activation(out=e8s[:], in_=psb[:], func=A.Exp,
                                     scale=0.8)

                # d cols: edd[:, 2jc] = exp(0.2 d), edd[:, 2jc+1] = exp(0.8 d)
                pdd = psmall.tile([P, 2 * NCH], f32, tag="sm", name="pdd")
                for jc in range(NCH):
                    nc.tensor.matmul(
                        out=pdd[:, 2 * jc:2 * jc + 2],
                        lhsT=tT[:, jc * P:(jc + 1) * P],
                        rhs=wpr[0:fo, c_ad + 2 * h:c_ad + 2 * h + 2],
                        start=True, stop=True,
                    )
                edd = ddpool.tile([P, 2 * NCH], f32, tag="edd")
                nc.scalar.activation(out=edd[:], in_=pdd[:], func=A.Exp)
                return h, wblks, e8s, edd

            def head_chunks(state):
                h, wblks, e8s, edd = state
                # attention chunks; Z column padded to partition 32 both
                # layers (PE base partitions must be 0/32/64)
                pat = pattn.tile([fon, N], f32, tag="pat")
                for jc in range(NCH):
                    # node-major h' for this chunk (+ ones column(s))
                    phn = psmall.tile([P, fon], f32, tag="sm", name="phn")
                    for k in range(kch):
                        nc.tensor.matmul(
                            out=phn[:, 0:fo],
                            lhsT=xT_in[k][:, jc * P:(jc + 1) * P],
                            rhs=wblks[k],
                            start=(k == 0),
                            stop=(k == kch - 1),
                        )
                    nc.tensor.matmul(
                        out=phn[:, fo:fon],
                        lhsT=wpb[0:1, CB_OR:CB_OR + P],
                        rhs=wpb[0:1, CB_O17:CB_O17 + (fon - fo)],
                        start=True, stop=True,
                    )
                    # ha' = phn * exp(0.2 d) (per-partition scale)
                    ha = hapool.tile([P, fon], bf16, tag="ha")
                    nc.scalar.activation(
                        out=ha[:], in_=phn[:], func=A.Identity,
                        scale=edd[:, 2 * jc:2 * jc + 1],
                    )
                    # C = max(exp(0.8 d) * E8S, 1)  (tensor_scalar, 4x mode)
                    cc = upool.tile([P, N], bf16, tag="cc", name="cc")
                    nc.vector.tensor_scalar(
                        out=cc[:], in0=e8s[:],
                        scalar1=edd[:, 2 * jc + 1:2 * jc + 2], scalar2=1.0,
                        op0=OP.mult, op1=OP.max,
                    )
                    # u = C * adjT  (tensor_tensor, 2x mode; some on POOL)
                    u = upool.tile([P, N], bf16, tag="u")
                    pool_jc = (2, 5, 7) if h % 2 else (2, 5)
                    ueng = nc.gpsimd if jc in pool_jc else nc.vector
                    ueng.tensor_tensor(
                        out=u[:], in0=cc[:],
                        in1=adjT[:, jc * N:(jc + 1) * N], op=OP.mult,
                    )
                    for hf in range(2):
                        fs = slice(hf * HALF, (hf + 1) * HALF)
                        nc.tensor.matmul(
                            out=pat[:, fs],
                            lhsT=ha[:],
                            rhs=u[:, fs],
                            start=(jc == 0),
                            stop=(jc == NCH - 1),
                        )
                # evacuate numerator + Z row
                ou = oupool.tile([fon, N], bf16, tag="ou")
                nc.scalar.activation(out=ou[:], in_=pat[:], func=A.Identity)
                ou_list.append(ou)

            # software-pipelined emission: head h+1's prologue is queued
            # before head h's chunk work so ACT (tanh/exp) runs ahead of
            # the DVE/PE chunk stream on the in-order engine queues
            prev = head_prologue(0)
            for h in range(1, H):
                nxt = head_prologue(h)
                head_chunks(prev)
                prev = nxt
            head_chunks(prev)

            # ---- layer epilogue: Z cols, reciprocal, normalize ----
            zcols = singles.tile([P, H * NCH], f32, tag=f"zcols{li}",
                                 name=f"zcols{li}")
            for c in range(NCH):
                pzc = psmall.tile([P, H], f32, tag="sm", name="pzc")
                for h in range(H):
                    nc.tensor.matmul(
                        out=pzc[:, h:h + 1],
                        lhsT=ou_list[h][32:33, c * P:(c + 1) * P],
                        rhs=zc_rhs,
                        start=True, stop=True,
                    )
                nc.vector.tensor_copy(out=zcols[:, c * H:(c + 1) * H],
                                      in_=pzc[:])
            rcols = singles.tile([P, H * NCH], f32, tag=f"rcols{li}",
                                 name=f"rcols{li}")
            rscr = singles.tile([P, H * NCH], f32, tag=f"rscr{li}",
                                name=f"rscr{li}")
            nc.vector.reciprocal_approx_accurate(
                out=rcols[:], in_=zcols[:], scratch=rscr[:]
            )
            rcolsb = singles.tile([P, H * NCH], bf16, tag=f"rcolsb{li}",
                                  name=f"rcolsb{li}")
            nc.vector.tensor_copy(out=rcolsb[:], in_=rcols[:])

            if li == 1:
                # node-major head-mean accumulator (reuses the pat bank)
                pfall = pattn.tile([P, P], f32, tag="pat", name="pfall")
            for h in range(H):
                ou = ou_list[h]
                # broadcast 1/Z rows: prb[o, c*128+p] = rcols[p, c*8+h]
                prb = pbig.tile([fo, N], f32, tag="big", name="prb")
                for c in range(NCH):
                    nc.tensor.matmul(
                        out=prb[:, c * P:(c + 1) * P],
                        lhsT=rcolsb[:, c * H + h:c * H + h + 1].to_broadcast(
                            [P, fo]),
                        rhs=identb[:],
                        start=True, stop=True,
                    )
                # normalize, parity-split so the layer-boundary epilogue
                # spreads across DVE / ACT+POOL
                if h % 2 == 0:
                    y = ypool.tile([fo, N], bf16, tag="y",
                                   name=f"y{li}", bufs=8)
                    nc.vector.scalar_tensor_tensor(
                        out=y[:], in0=ou[0:fo, :], scalar=1.0,
                        op0=OP.mult, in1=prb[:], op1=OP.mult,
                    )
                else:
                    rzb = ypool.tile([fo, N], bf16, tag="rzb", name="rzb")
                    nc.scalar.activation(out=rzb[:], in_=prb[:],
                                         func=A.Identity)
                    y = ypool.tile([fo, N], bf16, tag="y",
                                   name=f"y{li}", bufs=8)
                    nc.gpsimd.tensor_tensor(
                        out=y[:], in0=ou[0:fo, :], in1=rzb[:], op=OP.mult,
                    )
                if li == 0:
                    if not zero_b0:
                        yb = ypool.tile([fo, N], bf16, tag="y", name=f"y{li}",
                                        bufs=8)
                        nc.vector.tensor_scalar(
                            out=yb[:], in0=y[:],
                            scalar1=wp[0:fo, C_B0:C_B0 + 1],
                            scalar2=None, op0=OP.add,
                        )
                        y = yb
                    # x1 = elu(y) = max(min(exp(y),1)-1, y)
                    e = mpool.tile([fo, N], bf16, tag="e")
                    nc.scalar.activation(out=e[:], in_=y[:], func=A.Exp)
                    em1 = mpool.tile([fo, N], bf16, tag="em1")
                    nc.vector.tensor_scalar(
                        out=em1[:], in0=e[:], scalar1=1.0, scalar2=-1.0,
                        op0=OP.min, op1=OP.add,
                    )
                    k, j = divmod(h, 4)
                    nc.vector.tensor_tensor(
                        out=x1T[k][32 * j:32 * (j + 1), :], in0=em1[:],
                        in1=y[:], op=OP.max,
                    )
                else:
                    y_list.append(y)
            if li == 0:
                xT_in = x1T
            else:
                # head-mean folded into the final transpose: pfall chunk ic
                # accumulates (y_h chunk)^T over heads (the 1/8 rides the
                # 8*Z reciprocal).  One open psum group per zero region.
                for ic in range(NCH):
                    for h in range(H):
                        nc.tensor.matmul(
                            out=pfall[:, ic * 16:(ic + 1) * 16],
                            lhsT=y_list[h][:, ic * P:(ic + 1) * P],
                            rhs=identb[0:16, 0:16],
                            start=(h == 0),
                            stop=(h == H - 1),
                        )

        # ---- log_softmax over 16 features, node-major; batched by ACT
        # function so the table set never thrashes mid-epilogue ----
        fms, nmxs, ses = [], [], []
        for ic in range(NCH):
            fm = stg.tile([P, 16], f32, tag="fm", bufs=8)
            if zero_b1:
                nc.vector.tensor_copy(
                    out=fm[:], in_=pfall[:, ic * 16:(ic + 1) * 16]
                )
            else:
                nc.vector.tensor_tensor(
                    out=fm[:], in0=pfall[:, ic * 16:(ic + 1) * 16],
                    in1=wpb[:, CB_B1R:CB_B1R + 16], op=OP.add,
                )
            nmx = stg.tile([P, 1], f32, tag="nmx", bufs=8)
            nc.vector.tensor_reduce(
                out=nmx[:], in_=fm[:], axis=mybir.AxisListType.X,
                op=OP.max, negate=True,
            )
            et = stg.tile([P, 16], f32, tag="et")
            se = stg.tile([P, 1], f32, tag="se", bufs=8)
            nc.scalar.activation(
                out=et[:], in_=fm[:], func=A.Exp, bias=nmx[:, :1],
                accum_out=se[:, :1],
            )
            fms.append(fm)
            nmxs.append(nmx)
            ses.append(se)
        for ic in range(NCH):
            lse = stg.tile([P, 1], f32, tag="lse", bufs=4)
            nc.scalar.activation(out=lse[:], in_=ses[ic][:], func=A.Ln)
            res = stg.tile([P, 16], f32, tag="res", bufs=4)
            nc.vector.tensor_scalar(
                out=res[:], in0=fms[ic][:], scalar1=nmxs[ic][:, :1],
                scalar2=lse[:, :1], op0=OP.add, op1=OP.subtract,
            )
            nc.sync.dma_start(out=out_d[ic * P:(ic + 1) * P, :], in_=res[:])

    nc.compile()
    return nc


def _make_wpack(inputs):
    import ml_dtypes
    f32 = np.float32
    wpack = np.zeros((P, WCOLS), f32)
    wpack[:, C_IDENT:C_IDENT + P] = np.eye(P, dtype=f32)
    wpack[0:32, C_B0] = np.asarray(inputs["b0"], f32).reshape(32)
    wpack[0:16, C_B1] = np.asarray(inputs["b1"], f32).reshape(16)

    wpr = np.zeros((P, RCOLS), f32)
    w0 = np.asarray(inputs["w0"], f32)      # [8, 128, 32]
    for h in range(H):
        wpr[:, C_W0 + h * 32: C_W0 + (h + 1) * 32] = w0[h]
    w1 = np.asarray(inputs["w1"], f32)      # [8, 256, 16]
    for h in range(H):
        for k in range(2):
            wpr[:, C_W1 + (h * 2 + k) * 16: C_W1 + (h * 2 + k + 1) * 16] = \
                w1[h, k * P:(k + 1) * P, :]
    a_src0 = np.asarray(inputs["a_src0"], f32)[..., 0]  # [8, 32]
    a_dst0 = np.asarray(inputs["a_dst0"], f32)[..., 0]
    a_src1 = np.asarray(inputs["a_src1"], f32)[..., 0]  # [8, 16]
    a_dst1 = np.asarray(inputs["a_dst1"], f32)[..., 0]
    for h in range(H):
        wpr[0:32, C_AS0 + h] = a_src0[h]
        wpr[0:16, C_AS1 + h] = a_src1[h]
        wpr[0:32, C_AD0 + 2 * h] = 0.2 * a_dst0[h]
        wpr[0:32, C_AD0 + 2 * h + 1] = 0.8 * a_dst0[h]
        wpr[0:16, C_AD1 + 2 * h] = 0.2 * a_dst1[h]
        wpr[0:16, C_AD1 + 2 * h + 1] = 0.8 * a_dst1[h]

    wpbf = np.zeros((P, BCOLS), f32)
    wpbf[:, CB_ID:CB_ID + P] = np.eye(P, dtype=f32)
    wpbf[0, CB_OR:CB_OR + P] = 1.0
    wpbf[:, CB_ONE] = 1.0
    wpbf[:, CB_EIGHT] = 8.0
    wpbf[:, CB_B1R:CB_B1R + 16] = np.asarray(inputs["b1"], f32).reshape(1, 16)
    wpbf[0, CB_O17:CB_O17 + 17] = 1.0
    for h in range(H):
        for k in range(2):
            wpbf[:, CB_W1 + (h * 2 + k) * 16: CB_W1 + (h * 2 + k + 1) * 16] = \
                w1[h, k * P:(k + 1) * P, :]
    wpb = wpbf.astype(ml_dtypes.bfloat16)
    return wpack, wpr, wpb


def _prep_inputs(inputs):
    import ml_dtypes
    x = np.asarray(inputs["x"], np.float32)
    verts = np.asarray(inputs["vertices"]).astype(np.int32)
    adj = np.asarray(inputs["adj"])
    emb_w = np.ascontiguousarray(np.asarray(inputs["emb_w"], np.float32))
    wpack, wpr, wpb = _make_wpack(inputs)
    wpack = np.ascontiguousarray(wpack)
    wpr = np.ascontiguousarray(wpr)
    wpb = np.ascontiguousarray(wpb)
    in_maps = []
    for c in range(BS):
        in_maps.append({
            "xt": np.ascontiguousarray(x[c].T),
            "verts": np.ascontiguousarray(verts[c].reshape(NCH, P).T),
            "adjb": np.ascontiguousarray(adj[c].astype(ml_dtypes.bfloat16)),
            "emb_w": emb_w,
            "wpack": wpack,
            "wpackr": wpr,
            "wpackb": wpb,
        })
    zero_b0 = bool(np.all(np.asarray(inputs["b0"]) == 0))
    zero_b1 = bool(np.all(np.asarray(inputs["b1"]) == 0))
    return in_maps, zero_b0, zero_b1


def _run(inputs, trace=False):
    from concourse.bass_utils import run_bass_kernel_spmd

    in_maps, zero_b0, zero_b1 = _prep_inputs(inputs)
    key = ("prog", zero_b0, zero_b1)
    if key not in _CACHE:
        _CACHE[key] = _build(zero_b0, zero_b1)
    nc = _CACHE[key]
    res = run_bass_kernel_spmd(
        nc, in_maps, list(range(BS)), trace=trace
    )
    out = np.stack([res.results[c]["out"] for c in range(BS)], axis=0)
    return out.astype(np.float32), res


def kernel(**inputs):
    out, _ = _run(inputs, trace=False)
    return out


# revision 73
# speedup vs baseline: 1.0200x; 1.0200x over previous
"""BatchGAT Trainium2 kernel (Bass/Tile), data-parallel over the 8 subgraphs.

Per core (1 subgraph, n=1024 nodes, 8 heads, 2 GAT layers), the attention
matrix exp(leakyrelu(s_n + d_m)) is never exponentiated elementwise.
Using softmax's invariance to per-column (per-destination-node) scales:

  exp(lrelu(s+d)) = E2S[n] * max(exp(d_m)*exp(0.8 s_n), exp(0.2 d_m))

The E2S[n] column factor cancels between numerator and denominator, and
exp(d_m) is a per-partition (source node) scale folded into the matmul
lhsT (h_aug * exp(d)) during its PSUM evacuation.  What remains per
128x1024 attention chunk is ONE DVE op in the 4x perf mode:

  u = (E8S max exp(-0.8 d)_col) * adjT_chunk        (all bf16, SBUF)

followed by the bf16 numerator matmul (ones column scaled by exp(d)
yields the softmax denominator Z).  adj ships from the host as bf16 and
is transposed by the DMA crossbar (dma_start_transpose) straight into
SBUF.  x ships host-transposed; the embedding half of x0T is gathered
by indirect DMA and PE-transposed.  Normalization 1/Z is built in
column form (tiny PE gathers), reciprocal'd in one batched DVE op, and
broadcast back to rows via stride-0-lhsT matmuls against the bf16
identity; layer outputs (elu / head-mean) write their final stacked
layout directly as partition-sliced DVE/GPSIMD stores, so both layers
feed the next matmuls without restacking passes.
"""

import numpy as np

BS, N, VOCAB, EMB, FEAT = 8, 1024, 100000, 64, 64
P = 128
NCH = N // P  # 8 node chunks
H = 8
HALF = 512

# wpack (f32) columns
C_IDENT = 0            # [128,128] identity f32 (PE transpose helper)
C_B0 = C_IDENT + P     # 1 col, partitions 0..31
C_B1 = C_B0 + 1        # 1 col, partitions 0..15
WCOLS = C_B1 + 1

# wpackr (f32r) columns
C_W0 = 0               # 8 heads x 32 cols: w0[h] [128,32]
C_W1 = C_W0 + 8 * 32   # 8 heads x 2 kch x 16 cols: w1 blocks [128,16]
C_AS0 = C_W1 + 8 * 32  # 8 cols a_src0 (rows 0:32)
C_AS1 = C_AS0 + 8      # 8 cols a_src1 (rows 0:16)
C_AD0 = C_AS1 + 8      # 8 heads x 2 cols [a_dst0, -0.8*a_dst0] (rows 0:32)
C_AD1 = C_AD0 + 16     # 8 heads x 2 cols [a_dst1, -0.8*a_dst1] (rows 0:16)
RCOLS = C_AD1 + 16

# wpackb (bf16) columns
CB_ID = 0              # [128,128] identity bf16
CB_OR = CB_ID + P      # ones row: partition 0, 128 cols of 1.0
CB_ONE = CB_OR + P     # 1 col: 1.0 (partition 0)
CB_EIGHT = CB_ONE + 1  # 1 col: 8.0 (partition 0)
CB_B1R = CB_EIGHT + 1  # [128,16]: b1 broadcast to all partitions
CB_O17 = CB_B1R + 16   # 17 cols of 1.0 on partition 0
CB_W1 = CB_O17 + 17    # 8 heads x 2 kch x 16 cols: w1 blocks, bf16
BCOLS = CB_W1 + 8 * 2 * 16

_CACHE = {}


def _build(zero_b0, zero_b1):
    import concourse.bass as bass
    import concourse.tile as tile
    from concourse import bacc, mybir
    from contextlib import ExitStack

    dt = mybir.dt
    f32 = dt.float32
    f32r = dt.float32r
    bf16 = dt.bfloat16
    A = mybir.ActivationFunctionType
    OP = mybir.AluOpType

    nc = bacc.Bacc("TRN2", target_bir_lowering=False, debug=False,
                   dynamic_dma_scratch_size=65536)

    xt_d = nc.dram_tensor("xt", [FEAT, N], f32r, kind="ExternalInput")
    v_d = nc.dram_tensor("verts", [P, NCH], dt.int32, kind="ExternalInput")
    adjb_d = nc.dram_tensor("adjb", [N, N], bf16, kind="ExternalInput")
    emb_d = nc.dram_tensor("emb_w", [VOCAB, EMB], f32, kind="ExternalInput")
    wp_d = nc.dram_tensor("wpack", [P, WCOLS], f32, kind="ExternalInput")
    wpr_d = nc.dram_tensor("wpackr", [P, RCOLS], f32r, kind="ExternalInput")
    wpb_d = nc.dram_tensor("wpackb", [P, BCOLS], bf16, kind="ExternalInput")
    out_d = nc.dram_tensor("out", [N, 16], f32, kind="ExternalOutput")

    with tile.TileContext(nc) as tc, ExitStack() as ctx:
        singles = ctx.enter_context(tc.tile_pool(name="singles", bufs=1))
        eep = ctx.enter_context(tc.tile_pool(name="eep", bufs=8))
        hpool = ctx.enter_context(tc.tile_pool(name="hpool", bufs=2))
        epool = ctx.enter_context(tc.tile_pool(name="epool", bufs=2))
        ddpool = ctx.enter_context(tc.tile_pool(name="ddpool", bufs=4))
        hapool = ctx.enter_context(tc.tile_pool(name="hapool", bufs=6))
        upool = ctx.enter_context(tc.tile_pool(name="upool", bufs=6))
        oupool = ctx.enter_context(tc.tile_pool(name="oupool", bufs=8))
        ypool = ctx.enter_context(tc.tile_pool(name="ypool", bufs=2))
        mpool = ctx.enter_context(tc.tile_pool(name="mpool", bufs=2))
        stg = ctx.enter_context(tc.tile_pool(name="stg", bufs=3))
        pbig = ctx.enter_context(tc.tile_pool(name="pbig", bufs=2, space="PSUM"))
        pattn = ctx.enter_context(tc.tile_pool(name="pattn", bufs=1, space="PSUM"))
        psmall = ctx.enter_context(tc.tile_pool(name="psmall", bufs=2, space="PSUM"))

        # ---- inputs; order the critical x0T path before the adj transposes
        vts = singles.tile([P, NCH], dt.int32, tag="vts")
        nc.sync.dma_start(out=vts[:], in_=v_d[:, :])
        x0T = singles.tile([P, N], f32r, tag="x0T")
        nc.sync.dma_start(out=x0T[0:FEAT, :], in_=xt_d[:, :])
        wpr = singles.tile([P, RCOLS], f32r, tag="wpr")
        nc.sync.dma_start(out=wpr[:], in_=wpr_d[:, :])
        wpb = singles.tile([P, BCOLS], bf16, tag="wpb")
        nc.sync.dma_start(out=wpb[:], in_=wpb_d[:, :])
        identb = wpb[:, CB_ID:CB_ID + P]
        wp = singles.tile([P, WCOLS], f32, tag="wp")
        nc.sync.dma_start(out=wp[:], in_=wp_d[:, :])

        # ---- adjT via DMA crossbar transpose: adjT[p, jc*N+n] = adj[n, jc*128+p]
        adjT = singles.tile([P, NCH * N], bf16, tag="adjT")
        for jc in range(NCH):
            nc.sync.dma_start_transpose(
                out=adjT[:, jc * N:(jc + 1) * N],
                in_=adjb_d[:, jc * P:(jc + 1) * P],
            )

        # PE warmup
        pwarm = psmall.tile([16, 16], f32, tag="sm")
        nc.tensor.matmul(out=pwarm[:], lhsT=wpr[:, 0:16], rhs=wpr[:, 0:16],
                         start=True, stop=True)
        pwarm2 = psmall.tile([16, 16], f32, tag="sm", name="pwarm2")
        nc.tensor.matmul(out=pwarm2[:], lhsT=wpr[:, 0:16], rhs=wpr[:, 0:16],
                         start=True, stop=True)

        for c in range(NCH):
            ee = eep.tile([P, EMB], f32, tag="ee")
            nc.gpsimd.indirect_dma_start(
                out=ee[:],
                out_offset=None,
                in_=emb_d[:, :],
                in_offset=bass.IndirectOffsetOnAxis(ap=vts[:, c:c + 1], axis=0),
            )
            pe_t = psmall.tile([EMB, P], f32, tag="sm", name=f"pet{c}")
            nc.tensor.transpose(
                out=pe_t[:], in_=ee[:], identity=wp[:, C_IDENT:C_IDENT + P]
            )
            nc.vector.tensor_copy(
                out=x0T[FEAT:P, c * P:(c + 1) * P], in_=pe_t[:]
            )

        # ---- layers ----
        x1T = [
            singles.tile([P, N], bf16, tag=f"x1T{k}", name=f"x1T{k}")
            for k in range(2)
        ]
        xT_in = [x0T]
        fon = 33  # attn lhsT rows: fo outputs (+pad) + Z col at partition 32
        for li in range(2):
            fo = 32 if li == 0 else 16
            kch = 1 if li == 0 else 2
            c_w = C_W0 if li == 0 else C_W1
            wblk = 32 if li == 0 else 16
            c_as = C_AS0 if li == 0 else C_AS1
            c_ad = C_AD0 if li == 0 else C_AD1
            zc_rhs = wpb[32:33, CB_ONE:CB_ONE + 1] if li == 0 else \
                wpb[32:33, CB_EIGHT:CB_EIGHT + 1]
            ou_list = []
            y_list = []

            def head_prologue(h):
                # feature-major h' and tanh
                # weight blocks: L0 f32r (pairs with x0T f32r), L1 bf16
                # (pairs with x1T bf16; f32r may not mix with bf16)
                if li == 0:
                    wblks = [wpr[:, c_w + h * 32:c_w + h * 32 + fo]]
                else:
                    wblks = [
                        wpb[:, CB_W1 + (h * 2 + k) * 16:
                            CB_W1 + (h * 2 + k) * 16 + fo]
                        for k in range(2)
                    ]
                ph = pbig.tile([fo, N], f32, tag="big")
                for hf in range(2):
                    fs = slice(hf * HALF, (hf + 1) * HALF)
                    for k in range(kch):
                        nc.tensor.matmul(
                            out=ph[:, fs],
                            lhsT=wblks[k],
                            rhs=xT_in[k][:, fs],
                            start=(k == 0),
                            stop=(k == kch - 1),
                        )
                tT = hpool.tile([fo, N], f32r, tag="tT")
                nc.scalar.activation(out=tT[:], in_=ph[:], func=A.Tanh)

                # s broadcast -> E8S = exp(0.8 s) on all partitions (bf16)
                psb = pbig.tile([P, N], f32, tag="big", name="psb")
                for hf in range(2):
                    fs = slice(hf * HALF, (hf + 1) * HALF)
                    nc.tensor.matmul(
                        out=psb[:, fs],
                        lhsT=wpr[0:fo, c_as + h:c_as + h + 1].to_broadcast(
                            [fo, P]),
                        rhs=tT[:, fs],
                        start=True, stop=True,
                    )
                e8s = epool.tile([P, N], bf16, tag="e8s")
                nc.scalar.activation(out=e8s[:], in_=psb[:], func=A.Exp,
                                     scale=0.8)

                # d cols: edd[:, 2jc] = exp(0.2 d), edd[:, 2jc+1] = exp(0.8 d)
                pdd = psmall.tile([P, 2 * NCH], f32, tag="sm", name="pdd")
                for jc in range(NCH):
                    nc.tensor.matmul(
                        out=pdd[:, 2 * jc:2 * jc + 2],
                        lhsT=tT[:, jc * P:(jc + 1) * P],
                        rhs=wpr[0:fo, c_ad + 2 * h:c_ad + 2 * h + 2],
                        start=True, stop=True,
                    )
                edd = ddpool.tile([P, 2 * NCH], f32, tag="edd")
                nc.scalar.activation(out=edd[:], in_=pdd[:], func=A.Exp)
                return h, wblks, e8s, edd

            def head_chunks(state):
                h, wblks, e8s, edd = state
                # attention chunks; Z column padded to partition 32 both
                # layers (PE base partitions must be 0/32/64)
                pat = pattn.tile([fon, N], f32, tag="pat")
                for jc in range(NCH):
                    # node-major h' for this chunk (+ ones column(s))
                    phn = psmall.tile([P, fon], f32, tag="sm", name="phn")
                    for k in range(kch):
                        nc.tensor.matmul(
                            out=phn[:, 0:fo],
                            lhsT=xT_in[k][:, jc * P:(jc + 1) * P],
                            rhs=wblks[k],
                            start=(k == 0),
                            stop=(k == kch - 1),
                        )
                    nc.tensor.matmul(
                        out=phn[:, fo:fon],
                        lhsT=wpb[0:1, CB_OR:CB_OR + P],
                        rhs=wpb[0:1, CB_O17:CB_O17 + (fon - fo)],
                        start=True, stop=True,
                    )
                    # ha' = phn * exp(0.2 d) (per-partition scale)
                    ha = hapool.tile([P, fon], bf16, tag="ha")
                    nc.scalar.activation(
                        out=ha[:], in_=phn[:], func=A.Identity,
                        scale=edd[:, 2 * jc:2 * jc + 1],
                    )
                    # C = max(exp(0.8 d) * E8S, 1)  (tensor_scalar, 4x mode)
                    cc = upool.tile([P, N], bf16, tag="cc", name="cc")
                    nc.vector.tensor_scalar(
                        out=cc[:], in0=e8s[:],
                        scalar1=edd[:, 2 * jc + 1:2 * jc + 2], scalar2=1.0,
                        op0=OP.mult, op1=OP.max,
                    )
                    # u = C * adjT  (tensor_tensor, 2x mode; some on POOL)
                    u = upool.tile([P, N], bf16, tag="u")
                    pool_jc = (2, 5, 7) if h % 2 else (2, 5)
                    ueng = nc.gpsimd if jc in pool_jc else nc.vector
                    ueng.tensor_tensor(
                        out=u[:], in0=cc[:],
                        in1=adjT[:, jc * N:(jc + 1) * N], op=OP.mult,
                    )
                    for hf in range(2):
                        fs = slice(hf * HALF, (hf + 1) * HALF)
                        nc.tensor.matmul(
                            out=pat[:, fs],
                            lhsT=ha[:],
                            rhs=u[:, fs],
                            start=(jc == 0),
                            stop=(jc == NCH - 1),
                        )
                # evacuate numerator + Z row
                ou = oupool.tile([fon, N], bf16, tag="ou")
                if h % 2 == 0:
                    nc.scalar.activation(out=ou[:], in_=pat[:],
                                         func=A.Identity)
                else:
                    nc.vector.tensor_copy(out=ou[:], in_=pat[:])
                ou_list.append(ou)

            # software-pipelined emission: head h+1's prologue is queued
            # before head h's chunk work so ACT (tanh/exp) runs ahead of
            # the DVE/PE chunk stream on the in-order engine queues
            prev = head_prologue(0)
            for h in range(1, H):
                nxt = head_prologue(h)
                head_chunks(prev)
                prev = nxt
            head_chunks(prev)

            # ---- layer epilogue: Z cols, reciprocal, normalize ----
            zcols = singles.tile([P, H * NCH], f32, tag=f"zcols{li}",
                                 name=f"zcols{li}")
            for c in range(NCH):
                pzc = psmall.tile([P, H], f32, tag="sm", name="pzc")
                for h in range(H):
                    nc.tensor.matmul(
                        out=pzc[:, h:h + 1],
                        lhsT=ou_list[h][32:33, c * P:(c + 1) * P],
                        rhs=zc_rhs,
                        start=True, stop=True,
                    )
                nc.vector.tensor_copy(out=zcols[:, c * H:(c + 1) * H],
                                      in_=pzc[:])
            rcols = singles.tile([P, H * NCH], f32, tag=f"rcols{li}",
                                 name=f"rcols{li}")
            rscr = singles.tile([P, H * NCH], f32, tag=f"rscr{li}",
                                name=f"rscr{li}")
            nc.vector.reciprocal_approx_accurate(
                out=rcols[:], in_=zcols[:], scratch=rscr[:]
            )
            rcolsb = singles.tile([P, H * NCH], bf16, tag=f"rcolsb{li}",
                                  name=f"rcolsb{li}")
            nc.vector.tensor_copy(out=rcolsb[:], in_=rcols[:])

            if li == 1:
                # node-major head-mean accumulator (reuses the pat bank)
                pfall = pattn.tile([P, P], f32, tag="pat", name="pfall")
            for h in range(H):
                ou = ou_list[h]
                # broadcast 1/Z rows: prb[o, c*128+p] = rcols[p, c*8+h]
                prb = pbig.tile([fo, N], f32, tag="big", name="prb")
                for c in range(NCH):
                    nc.tensor.matmul(
                        out=prb[:, c * P:(c + 1) * P],
                        lhsT=rcolsb[:, c * H + h:c * H + h + 1].to_broadcast(
                            [P, fo]),
                        rhs=identb[:],
                        start=True, stop=True,
                    )
                # normalize, parity-split so the layer-boundary epilogue
                # spreads across DVE / ACT+POOL
                if h % 2 == 0:
                    y = ypool.tile([fo, N], bf16, tag="y",
                                   name=f"y{li}", bufs=8)
                    nc.vector.scalar_tensor_tensor(
                        out=y[:], in0=ou[0:fo, :], scalar=1.0,
                        op0=OP.mult, in1=prb[:], op1=OP.mult,
                    )
                else:
                    rzb = ypool.tile([fo, N], bf16, tag="rzb", name="rzb")
                    nc.scalar.activation(out=rzb[:], in_=prb[:],
                                         func=A.Identity)
                    y = ypool.tile([fo, N], bf16, tag="y",
                                   name=f"y{li}", bufs=8)
                    nc.gpsimd.tensor_tensor(
                        out=y[:], in0=ou[0:fo, :], in1=rzb[:], op=OP.mult,
                    )
                if li == 0:
                    if not zero_b0:
                        yb = ypool.tile([fo, N], bf16, tag="y", name=f"y{li}",
                                        bufs=8)
                        nc.vector.tensor_scalar(
                            out=yb[:], in0=y[:],
                            scalar1=wp[0:fo, C_B0:C_B0 + 1],
                            scalar2=None, op0=OP.add,
                        )
                        y = yb
                    # x1 = elu(y) = max(min(exp(y),1)-1, y)
                    e = mpool.tile([fo, N], bf16, tag="e")
                    nc.scalar.activation(out=e[:], in_=y[:], func=A.Exp)
                    em1 = mpool.tile([fo, N], bf16, tag="em1")
                    nc.vector.tensor_scalar(
                        out=em1[:], in0=e[:], scalar1=1.0, scalar2=-1.0,
                        op0=OP.min, op1=OP.add,
                    )
                    k, j = divmod(h, 4)
                    nc.vector.tensor_tensor(
                        out=x1T[k][32 * j:32 * (j + 1), :], in0=em1[:],
                        in1=y[:], op=OP.max,
                    )
                else:
                    y_list.append(y)
            if li == 0:
                xT_in = x1T
            else:
                # head-mean folded into the final transpose: pfall chunk ic
                # accumulates (y_h chunk)^T over heads (the 1/8 rides the
                # 8*Z reciprocal).  One open psum group per zero region.
                for ic in range(NCH):
                    for h in range(H):
                        nc.tensor.matmul(
                            out=pfall[:, ic * 16:(ic + 1) * 16],
                            lhsT=y_list[h][:, ic * P:(ic + 1) * P],
                            rhs=identb[0:16, 0:16],
                            start=(h == 0),
                            stop=(h == H - 1),
                        )

        # ---- log_softmax over 16 features, node-major; batched by ACT
        # function so the table set never thrashes mid-epilogue ----
        fms, nmxs, ses = [], [], []
        for ic in range(NCH):
            fm = stg.tile([P, 16], f32, tag="fm", bufs=8)
            if zero_b1:
                nc.vector.tensor_copy(
                    out=fm[:], in_=pfall[:, ic * 16:(ic + 1) * 16]
                )
            else:
                nc.vector.tensor_tensor(
                    out=fm[:], in0=pfall[:, ic * 16:(ic + 1) * 16],
                    in1=wpb[:, CB_B1R:CB_B1R + 16], op=OP.add,
                )
            nmx = stg.tile([P, 1], f32, tag="nmx", bufs=8)
            nc.vector.tensor_reduce(
                out=nmx[:], in_=fm[:], axis=mybir.AxisListType.X,
                op=OP.max, negate=True,
            )
            et = stg.tile([P, 16], f32, tag="et")
            se = stg.tile([P, 1], f32, tag="se", bufs=8)
            nc.scalar.activation(
                out=et[:], in_=fm[:], func=A.Exp, bias=nmx[:, :1],
                accum_out=se[:, :1],
            )
            fms.append(fm)
            nmxs.append(nmx)
            ses.append(se)
        for ic in range(NCH):
            lse = stg.tile([P, 1], f32, tag="lse", bufs=4)
            nc.scalar.activation(out=lse[:], in_=ses[ic][:], func=A.Ln)
            res = stg.tile([P, 16], f32, tag="res", bufs=4)
            nc.vector.tensor_scalar(
                out=res[:], in0=fms[ic][:], scalar1=nmxs[ic][:, :1],
                scalar2=lse[:, :1], op0=OP.add, op1=OP.subtract,
            )
            nc.sync.dma_start(out=out_d[ic * P:(ic + 1) * P, :], in_=res[:])

    nc.compile()
    return nc


def _make_wpack(inputs):
    import ml_dtypes
    f32 = np.float32
    wpack = np.zeros((P, WCOLS), f32)
    wpack[:, C_IDENT:C_IDENT + P] = np.eye(P, dtype=f32)
    wpack[0:32, C_B0] = np.asarray(inputs["b0"], f32).reshape(32)
    wpack[0:16, C_B1] = np.asarray(inputs["b1"], f32).reshape(16)

    wpr = np.zeros((P, RCOLS), f32)
    w0 = np.asarray(inputs["w0"], f32)      # [8, 128, 32]
    for h in range(H):
        wpr[:, C_W0 + h * 32: C_W0 + (h + 1) * 32] = w0[h]
    w1 = np.asarray(inputs["w1"], f32)      # [8, 256, 16]
    for h in range(H):
        for k in range(2):
            wpr[:, C_W1 + (h * 2 + k) * 16: C_W1 + (h * 2 + k + 1) * 16] = \
                w1[h, k * P:(k + 1) * P, :]
    a_src0 = np.asarray(inputs["a_src0"], f32)[..., 0]  # [8, 32]
    a_dst0 = np.asarray(inputs["a_dst0"], f32)[..., 0]
    a_src1 = np.asarray(inputs["a_src1"], f32)[..., 0]  # [8, 16]
    a_dst1 = np.asarray(inputs["a_dst1"], f32)[..., 0]
    for h in range(H):
        wpr[0:32, C_AS0 + h] = a_src0[h]
        wpr[0:16, C_AS1 + h] = a_src1[h]
        wpr[0:32, C_AD0 + 2 * h] = 0.2 * a_dst0[h]
        wpr[0:32, C_AD0 + 2 * h + 1] = 0.8 * a_dst0[h]
        wpr[0:16, C_AD1 + 2 * h] = 0.2 * a_dst1[h]
        wpr[0:16, C_AD1 + 2 * h + 1] = 0.8 * a_dst1[h]

    wpbf = np.zeros((P, BCOLS), f32)
    wpbf[:, CB_ID:CB_ID + P] = np.eye(P, dtype=f32)
    wpbf[0, CB_OR:CB_OR + P] = 1.0
    wpbf[:, CB_ONE] = 1.0
    wpbf[:, CB_EIGHT] = 8.0
    wpbf[:, CB_B1R:CB_B1R + 16] = np.asarray(inputs["b1"], f32).reshape(1, 16)
    wpbf[0, CB_O17:CB_O17 + 17] = 1.0
    for h in range(H):
        for k in range(2):
            wpbf[:, CB_W1 + (h * 2 + k) * 16: CB_W1 + (h * 2 + k + 1) * 16] = \
                w1[h, k * P:(k + 1) * P, :]
    wpb = wpbf.astype(ml_dtypes.bfloat16)
    return wpack, wpr, wpb


def _prep_inputs(inputs):
    import ml_dtypes
    x = np.asarray(inputs["x"], np.float32)
    verts = np.asarray(inputs["vertices"]).astype(np.int32)
    adj = np.asarray(inputs["adj"])
    emb_w = np.ascontiguousarray(np.asarray(inputs["emb_w"], np.float32))
    wpack, wpr, wpb = _make_wpack(inputs)
    wpack = np.ascontiguousarray(wpack)
    wpr = np.ascontiguousarray(wpr)
    wpb = np.ascontiguousarray(wpb)
    in_maps = []
    for c in range(BS):
        in_maps.append({
            "xt": np.ascontiguousarray(x[c].T),
            "verts": np.ascontiguousarray(verts[c].reshape(NCH, P).T),
            "adjb": np.ascontiguousarray(adj[c].astype(ml_dtypes.bfloat16)),
            "emb_w": emb_w,
            "wpack": wpack,
            "wpackr": wpr,
            "wpackb": wpb,
        })
    zero_b0 = bool(np.all(np.asarray(inputs["b0"]) == 0))
    zero_b1 = bool(np.all(np.asarray(inputs["b1"]) == 0))
    return in_maps, zero_b0, zero_b1


def _run(inputs, trace=False):
    from concourse.bass_utils import run_bass_kernel_spmd

    in_maps, zero_b0, zero_b1 = _prep_inputs(inputs)
    key = ("prog", zero_b0, zero_b1)
    if key not in _CACHE:
        _CACHE[key] = _build(zero_b0, zero_b1)
    nc = _CACHE[key]
    res = run_bass_kernel_spmd(
        nc, in_maps, list(range(BS)), trace=trace
    )
    out = np.stack([res.results[c]["out"] for c in range(BS)], axis=0)
    return out.astype(np.float32), res


def kernel(**inputs):
    out, _ = _run(inputs, trace=False)
    return out


# revision 74
# speedup vs baseline: 1.0276x; 1.0074x over previous
"""BatchGAT Trainium2 kernel (Bass/Tile), data-parallel over the 8 subgraphs.

Per core (1 subgraph, n=1024 nodes, 8 heads, 2 GAT layers), the attention
matrix exp(leakyrelu(s_n + d_m)) is never exponentiated elementwise.
Using softmax's invariance to per-column (per-destination-node) scales:

  exp(lrelu(s+d)) = E2S[n] * max(exp(d_m)*exp(0.8 s_n), exp(0.2 d_m))

The E2S[n] column factor cancels between numerator and denominator, and
exp(d_m) is a per-partition (source node) scale folded into the matmul
lhsT (h_aug * exp(d)) during its PSUM evacuation.  What remains per
128x1024 attention chunk is ONE DVE op in the 4x perf mode:

  u = (E8S max exp(-0.8 d)_col) * adjT_chunk        (all bf16, SBUF)

followed by the bf16 numerator matmul (ones column scaled by exp(d)
yields the softmax denominator Z).  adj ships from the host as bf16 and
is transposed by the DMA crossbar (dma_start_transpose) straight into
SBUF.  x ships host-transposed; the embedding half of x0T is gathered
by indirect DMA and PE-transposed.  Normalization 1/Z is built in
column form (tiny PE gathers), reciprocal'd in one batched DVE op, and
broadcast back to rows via stride-0-lhsT matmuls against the bf16
identity; layer outputs (elu / head-mean) write their final stacked
layout directly as partition-sliced DVE/GPSIMD stores, so both layers
feed the next matmuls without restacking passes.
"""

import numpy as np

BS, N, VOCAB, EMB, FEAT = 8, 1024, 100000, 64, 64
P = 128
NCH = N // P  # 8 node chunks
H = 8
HALF = 512

# wpack (f32) columns
C_IDENT = 0            # [128,128] identity f32 (PE transpose helper)
C_B0 = C_IDENT + P     # 1 col, partitions 0..31
C_B1 = C_B0 + 1        # 1 col, partitions 0..15
WCOLS = C_B1 + 1

# wpackr (f32r) columns
C_W0 = 0               # 8 heads x 32 cols: w0[h] [128,32]
C_W1 = C_W0 + 8 * 32   # 8 heads x 2 kch x 16 cols: w1 blocks [128,16]
C_AS0 = C_W1 + 8 * 32  # 8 cols a_src0 (rows 0:32)
C_AS1 = C_AS0 + 8      # 8 cols a_src1 (rows 0:16)
C_AD0 = C_AS1 + 8      # 8 heads x 2 cols [a_dst0, -0.8*a_dst0] (rows 0:32)
C_AD1 = C_AD0 + 16     # 8 heads x 2 cols [a_dst1, -0.8*a_dst1] (rows 0:16)
RCOLS = C_AD1 + 16

# wpackb (bf16) columns
CB_ID = 0              # [128,128] identity bf16
CB_OR = CB_ID + P      # ones row: partition 0, 128 cols of 1.0
CB_ONE = CB_OR + P     # 1 col: 1.0 (partition 0)
CB_EIGHT = CB_ONE + 1  # 1 col: 8.0 (partition 0)
CB_B1R = CB_EIGHT + 1  # [128,16]: b1 broadcast to all partitions
CB_O17 = CB_B1R + 16   # 17 cols of 1.0 on partition 0
CB_W1 = CB_O17 + 17    # 8 heads x 2 kch x 16 cols: w1 blocks, bf16
BCOLS = CB_W1 + 8 * 2 * 16

_CACHE = {}


def _build(zero_b0, zero_b1):
    import concourse.bass as bass
    import concourse.tile as tile
    from concourse import bacc, mybir
    from contextlib import ExitStack

    dt = mybir.dt
    f32 = dt.float32
    f32r = dt.float32r
    bf16 = dt.bfloat16
    A = mybir.ActivationFunctionType
    OP = mybir.AluOpType

    nc = bacc.Bacc("TRN2", target_bir_lowering=False, debug=False,
                   dynamic_dma_scratch_size=65536)

    xt_d = nc.dram_tensor("xt", [FEAT, N], f32r, kind="ExternalInput")
    v_d = nc.dram_tensor("verts", [P, NCH], dt.int32, kind="ExternalInput")
    adjb_d = nc.dram_tensor("adjb", [N, N], bf16, kind="ExternalInput")
    emb_d = nc.dram_tensor("emb_w", [VOCAB, EMB], f32, kind="ExternalInput")
    wp_d = nc.dram_tensor("wpack", [P, WCOLS], f32, kind="ExternalInput")
    wpr_d = nc.dram_tensor("wpackr", [P, RCOLS], f32r, kind="ExternalInput")
    wpb_d = nc.dram_tensor("wpackb", [P, BCOLS], bf16, kind="ExternalInput")
    out_d = nc.dram_tensor("out", [N, 16], f32, kind="ExternalOutput")

    with tile.TileContext(nc) as tc, ExitStack() as ctx:
        singles = ctx.enter_context(tc.tile_pool(name="singles", bufs=1))
        eep = ctx.enter_context(tc.tile_pool(name="eep", bufs=8))
        hpool = ctx.enter_context(tc.tile_pool(name="hpool", bufs=2))
        epool = ctx.enter_context(tc.tile_pool(name="epool", bufs=2))
        ddpool = ctx.enter_context(tc.tile_pool(name="ddpool", bufs=4))
        hapool = ctx.enter_context(tc.tile_pool(name="hapool", bufs=6))
        upool = ctx.enter_context(tc.tile_pool(name="upool", bufs=6))
        oupool = ctx.enter_context(tc.tile_pool(name="oupool", bufs=8))
        ypool = ctx.enter_context(tc.tile_pool(name="ypool", bufs=2))
        mpool = ctx.enter_context(tc.tile_pool(name="mpool", bufs=2))
        stg = ctx.enter_context(tc.tile_pool(name="stg", bufs=3))
        pbig = ctx.enter_context(tc.tile_pool(name="pbig", bufs=2, space="PSUM"))
        pattn = ctx.enter_context(tc.tile_pool(name="pattn", bufs=1, space="PSUM"))
        psmall = ctx.enter_context(tc.tile_pool(name="psmall", bufs=2, space="PSUM"))

        # ---- inputs; order the critical x0T path before the adj transposes
        vts = singles.tile([P, NCH], dt.int32, tag="vts")
        nc.sync.dma_start(out=vts[:], in_=v_d[:, :])
        x0T = singles.tile([P, N], f32r, tag="x0T")
        nc.sync.dma_start(out=x0T[0:FEAT, :], in_=xt_d[:, :])
        wpr = singles.tile([P, RCOLS], f32r, tag="wpr")
        nc.sync.dma_start(out=wpr[:], in_=wpr_d[:, :])
        wpb = singles.tile([P, BCOLS], bf16, tag="wpb")
        nc.sync.dma_start(out=wpb[:], in_=wpb_d[:, :])
        identb = wpb[:, CB_ID:CB_ID + P]
        wp = singles.tile([P, WCOLS], f32, tag="wp")
        nc.sync.dma_start(out=wp[:], in_=wp_d[:, :])

        # ---- adjT via DMA crossbar transpose: adjT[p, jc*N+n] = adj[n, jc*128+p]
        adjT = singles.tile([P, NCH * N], bf16, tag="adjT")
        for jc in range(NCH):
            nc.sync.dma_start_transpose(
                out=adjT[:, jc * N:(jc + 1) * N],
                in_=adjb_d[:, jc * P:(jc + 1) * P],
            )

        # PE warmup
        pwarm = psmall.tile([16, 16], f32, tag="sm")
        nc.tensor.matmul(out=pwarm[:], lhsT=wpr[:, 0:16], rhs=wpr[:, 0:16],
                         start=True, stop=True)
        pwarm2 = psmall.tile([16, 16], f32, tag="sm", name="pwarm2")
        nc.tensor.matmul(out=pwarm2[:], lhsT=wpr[:, 0:16], rhs=wpr[:, 0:16],
                         start=True, stop=True)

        for c in range(NCH):
            ee = eep.tile([P, EMB], f32, tag="ee")
            nc.gpsimd.indirect_dma_start(
                out=ee[:],
                out_offset=None,
                in_=emb_d[:, :],
                in_offset=bass.IndirectOffsetOnAxis(ap=vts[:, c:c + 1], axis=0),
            )
            pe_t = psmall.tile([EMB, P], f32, tag="sm", name=f"pet{c}")
            nc.tensor.transpose(
                out=pe_t[:], in_=ee[:], identity=wp[:, C_IDENT:C_IDENT + P]
            )
            nc.vector.tensor_copy(
                out=x0T[FEAT:P, c * P:(c + 1) * P], in_=pe_t[:]
            )

        # ---- layers ----
        x1T = [
            singles.tile([P, N], bf16, tag=f"x1T{k}", name=f"x1T{k}")
            for k in range(2)
        ]
        xT_in = [x0T]
        fon = 33  # attn lhsT rows: fo outputs (+pad) + Z col at partition 32
        for li in range(2):
            fo = 32 if li == 0 else 16
            kch = 1 if li == 0 else 2
            c_w = C_W0 if li == 0 else C_W1
            wblk = 32 if li == 0 else 16
            c_as = C_AS0 if li == 0 else C_AS1
            c_ad = C_AD0 if li == 0 else C_AD1
            zc_rhs = wpb[32:33, CB_ONE:CB_ONE + 1] if li == 0 else \
                wpb[32:33, CB_EIGHT:CB_EIGHT + 1]
            ou_list = []
            y_list = []

            def head_prologue(h):
                # feature-major h' and tanh
                # weight blocks: L0 f32r (pairs with x0T f32r), L1 bf16
                # (pairs with x1T bf16; f32r may not mix with bf16)
                if li == 0:
                    wblks = [wpr[:, c_w + h * 32:c_w + h * 32 + fo]]
                else:
                    wblks = [
                        wpb[:, CB_W1 + (h * 2 + k) * 16:
                            CB_W1 + (h * 2 + k) * 16 + fo]
                        for k in range(2)
                    ]
                ph = pbig.tile([fo, N], f32, tag="big")
                for hf in range(2):
                    fs = slice(hf * HALF, (hf + 1) * HALF)
                    for k in range(kch):
                        nc.tensor.matmul(
                            out=ph[:, fs],
                            lhsT=wblks[k],
                            rhs=xT_in[k][:, fs],
                            start=(k == 0),
                            stop=(k == kch - 1),
                        )
                tT = hpool.tile([fo, N], f32r, tag="tT")
                nc.scalar.activation(out=tT[:], in_=ph[:], func=A.Tanh)

                # s broadcast -> E8S = exp(0.8 s) on all partitions (bf16)
                psb = pbig.tile([P, N], f32, tag="big", name="psb")
                for hf in range(2):
                    fs = slice(hf * HALF, (hf + 1) * HALF)
                    nc.tensor.matmul(
                        out=psb[:, fs],
                        lhsT=wpr[0:fo, c_as + h:c_as + h + 1].to_broadcast(
                            [fo, P]),
                        rhs=tT[:, fs],
                        start=True, stop=True,
                    )
                e8s = epool.tile([P, N], bf16, tag="e8s")
                nc.scalar.activation(out=e8s[:], in_=psb[:], func=A.Exp,
                                     scale=0.8)

                # d cols: edd[:, 2jc] = exp(0.2 d), edd[:, 2jc+1] = exp(0.8 d)
                pdd = psmall.tile([P, 2 * NCH], f32, tag="sm", name="pdd")
                for jc in range(NCH):
                    nc.tensor.matmul(
                        out=pdd[:, 2 * jc:2 * jc + 2],
                        lhsT=tT[:, jc * P:(jc + 1) * P],
                        rhs=wpr[0:fo, c_ad + 2 * h:c_ad + 2 * h + 2],
                        start=True, stop=True,
                    )
                edd = ddpool.tile([P, 2 * NCH], f32, tag="edd")
                nc.scalar.activation(out=edd[:], in_=pdd[:], func=A.Exp)
                return h, wblks, e8s, edd

            def head_chunks(state):
                h, wblks, e8s, edd = state
                # attention chunks; Z column padded to partition 32 both
                # layers (PE base partitions must be 0/32/64)
                pat = pattn.tile([fon, N], f32, tag="pat")
                for jc in range(NCH):
                    # node-major h' for this chunk (+ ones column(s))
                    phn = psmall.tile([P, fon], f32, tag="sm", name="phn")
                    for k in range(kch):
                        nc.tensor.matmul(
                            out=phn[:, 0:fo],
                            lhsT=xT_in[k][:, jc * P:(jc + 1) * P],
                            rhs=wblks[k],
                            start=(k == 0),
                            stop=(k == kch - 1),
                        )
                    nc.tensor.matmul(
                        out=phn[:, fo:fon],
                        lhsT=wpb[0:1, CB_OR:CB_OR + P],
                        rhs=wpb[0:1, CB_O17:CB_O17 + (fon - fo)],
                        start=True, stop=True,
                    )
                    # ha' = phn * exp(0.2 d) (per-partition scale)
                    ha = hapool.tile([P, fon], bf16, tag="ha")
                    nc.scalar.activation(
                        out=ha[:], in_=phn[:], func=A.Identity,
                        scale=edd[:, 2 * jc:2 * jc + 1],
                    )
                    # C = max(exp(0.8 d) * E8S, 1)  (tensor_scalar, 4x mode)
                    cc = upool.tile([P, N], bf16, tag="cc", name="cc")
                    nc.vector.tensor_scalar(
                        out=cc[:], in0=e8s[:],
                        scalar1=edd[:, 2 * jc + 1:2 * jc + 2], scalar2=1.0,
                        op0=OP.mult, op1=OP.max,
                    )
                    # u = C * adjT  (tensor_tensor, 2x mode; some on POOL)
                    u = upool.tile([P, N], bf16, tag="u")
                    ueng = nc.gpsimd if jc in (2, 5) else nc.vector
                    ueng.tensor_tensor(
                        out=u[:], in0=cc[:],
                        in1=adjT[:, jc * N:(jc + 1) * N], op=OP.mult,
                    )
                    for hf in range(2):
                        fs = slice(hf * HALF, (hf + 1) * HALF)
                        nc.tensor.matmul(
                            out=pat[:, fs],
                            lhsT=ha[:],
                            rhs=u[:, fs],
                            start=(jc == 0),
                            stop=(jc == NCH - 1),
                        )
                # evacuate numerator + Z row
                ou = oupool.tile([fon, N], bf16, tag="ou")
                if h % 2 == 0:
                    nc.scalar.activation(out=ou[:], in_=pat[:],
                                         func=A.Identity)
                else:
                    nc.vector.tensor_copy(out=ou[:], in_=pat[:])
                ou_list.append(ou)

            # software-pipelined emission: head h+1's prologue is queued
            # before head h's chunk work so ACT (tanh/exp) runs ahead of
            # the DVE/PE chunk stream on the in-order engine queues
            prev = head_prologue(0)
            for h in range(1, H):
                nxt = head_prologue(h)
                head_chunks(prev)
                prev = nxt
            head_chunks(prev)

            # ---- layer epilogue: Z cols, reciprocal, normalize ----
            zcols = singles.tile([P, H * NCH], f32, tag=f"zcols{li}",
                                 name=f"zcols{li}")
            for c in range(NCH):
                pzc = psmall.tile([P, H], f32, tag="sm", name="pzc")
                for h in range(H):
                    nc.tensor.matmul(
                        out=pzc[:, h:h + 1],
                        lhsT=ou_list[h][32:33, c * P:(c + 1) * P],
                        rhs=zc_rhs,
                        start=True, stop=True,
                    )
                nc.vector.tensor_copy(out=zcols[:, c * H:(c + 1) * H],
                                      in_=pzc[:])
            rcols = singles.tile([P, H * NCH], f32, tag=f"rcols{li}",
                                 name=f"rcols{li}")
            rscr = singles.tile([P, H * NCH], f32, tag=f"rscr{li}",
                                name=f"rscr{li}")
            nc.vector.reciprocal_approx_accurate(
                out=rcols[:], in_=zcols[:], scratch=rscr[:]
            )
            rcolsb = singles.tile([P, H * NCH], bf16, tag=f"rcolsb{li}",
                                  name=f"rcolsb{li}")
            nc.vector.tensor_copy(out=rcolsb[:], in_=rcols[:])

            if li == 1:
                # node-major head-mean accumulator (reuses the pat bank)
                pfall = pattn.tile([P, P], f32, tag="pat", name="pfall")
            for h in range(H):
                ou = ou_list[h]
                # broadcast 1/Z rows: prb[o, c*128+p] = rcols[p, c*8+h]
                prb = pbig.tile([fo, N], f32, tag="big", name="prb")
                for c in range(NCH):
                    nc.tensor.matmul(
                        out=prb[:, c * P:(c + 1) * P],
                        lhsT=rcolsb[:, c * H + h:c * H + h + 1].to_broadcast(
                            [P, fo]),
                        rhs=identb[:],
                        start=True, stop=True,
                    )
                # normalize, parity-split so the layer-boundary epilogue
                # spreads across DVE / ACT+POOL
                if h % 2 == 0:
                    y = ypool.tile([fo, N], bf16, tag="y",
                                   name=f"y{li}", bufs=8)
                    nc.vector.scalar_tensor_tensor(
                        out=y[:], in0=ou[0:fo, :], scalar=1.0,
                        op0=OP.mult, in1=prb[:], op1=OP.mult,
                    )
                else:
                    rzb = ypool.tile([fo, N], bf16, tag="rzb", name="rzb")
                    nc.scalar.activation(out=rzb[:], in_=prb[:],
                                         func=A.Identity)
                    y = ypool.tile([fo, N], bf16, tag="y",
                                   name=f"y{li}", bufs=8)
                    nc.gpsimd.tensor_tensor(
                        out=y[:], in0=ou[0:fo, :], in1=rzb[:], op=OP.mult,
                    )
                if li == 0:
                    if not zero_b0:
                        yb = ypool.tile([fo, N], bf16, tag="y", name=f"y{li}",
                                        bufs=8)
                        nc.vector.tensor_scalar(
                            out=yb[:], in0=y[:],
                            scalar1=wp[0:fo, C_B0:C_B0 + 1],
                            scalar2=None, op0=OP.add,
                        )
                        y = yb
                    # x1 = elu(y) = max(min(exp(y),1)-1, y)
                    e = mpool.tile([fo, N], bf16, tag="e")
                    nc.scalar.activation(out=e[:], in_=y[:], func=A.Exp)
                    em1 = mpool.tile([fo, N], bf16, tag="em1")
                    nc.vector.tensor_scalar(
                        out=em1[:], in0=e[:], scalar1=1.0, scalar2=-1.0,
                        op0=OP.min, op1=OP.add,
                    )
                    k, j = divmod(h, 4)
                    nc.vector.tensor_tensor(
                        out=x1T[k][32 * j:32 * (j + 1), :], in0=em1[:],
                        in1=y[:], op=OP.max,
                    )
                else:
                    y_list.append(y)
            if li == 0:
                xT_in = x1T
            else:
                # head-mean folded into the final transpose: pfall chunk ic
                # accumulates (y_h chunk)^T over heads (the 1/8 rides the
                # 8*Z reciprocal).  One open psum group per zero region.
                for ic in range(NCH):
                    for h in range(H):
                        nc.tensor.matmul(
                            out=pfall[:, ic * 16:(ic + 1) * 16],
                            lhsT=y_list[h][:, ic * P:(ic + 1) * P],
                            rhs=identb[0:16, 0:16],
                            start=(h == 0),
                            stop=(h == H - 1),
                        )

        # ---- log_softmax over 16 features, node-major; batched by ACT
        # function so the table set never thrashes mid-epilogue ----
        fms, nmxs, ses = [], [], []
        for ic in range(NCH):
            fm = stg.tile([P, 16], f32, tag="fm", bufs=8)
            if zero_b1:
                nc.vector.tensor_copy(
                    out=fm[:], in_=pfall[:, ic * 16:(ic + 1) * 16]
                )
            else:
                nc.vector.tensor_tensor(
                    out=fm[:], in0=pfall[:, ic * 16:(ic + 1) * 16],
                    in1=wpb[:, CB_B1R:CB_B1R + 16], op=OP.add,
                )
            nmx = stg.tile([P, 1], f32, tag="nmx", bufs=8)
            nc.vector.tensor_reduce(
                out=nmx[:], in_=fm[:], axis=mybir.AxisListType.X,
                op=OP.max, negate=True,
            )
            et = stg.tile([P, 16], f32, tag="et")
            se = stg.tile([P, 1], f32, tag="se", bufs=8)
            nc.scalar.activation(
                out=et[:], in_=fm[:], func=A.Exp, bias=nmx[:, :1],
                accum_out=se[:, :1],
            )
            fms.append(fm)
            nmxs.append(nmx)
            ses.append(se)
        for ic in range(NCH):
            lse = stg.tile([P, 1], f32, tag="lse", bufs=4)
            nc.scalar.activation(out=lse[:], in_=ses[ic][:], func=A.Ln)
            res = stg.tile([P, 16], f32, tag="res", bufs=4)
            nc.vector.tensor_scalar(
                out=res[:], in0=fms[ic][:], scalar1=nmxs[ic][:, :1],
                scalar2=lse[:, :1], op0=OP.add, op1=OP.subtract,
            )
            nc.sync.dma_start(out=out_d[ic * P:(ic + 1) * P, :], in_=res[:])

    nc.compile()
    return nc


def _make_wpack(inputs):
    import ml_dtypes
    f32 = np.float32
    wpack = np.zeros((P, WCOLS), f32)
    wpack[:, C_IDENT:C_IDENT + P] = np.eye(P, dtype=f32)
    wpack[0:32, C_B0] = np.asarray(inputs["b0"], f32).reshape(32)
    wpack[0:16, C_B1] = np.asarray(inputs["b1"], f32).reshape(16)

    wpr = np.zeros((P, RCOLS), f32)
    w0 = np.asarray(inputs["w0"], f32)      # [8, 128, 32]
    for h in range(H):
        wpr[:, C_W0 + h * 32: C_W0 + (h + 1) * 32] = w0[h]
    w1 = np.asarray(inputs["w1"], f32)      # [8, 256, 16]
    for h in range(H):
        for k in range(2):
            wpr[:, C_W1 + (h * 2 + k) * 16: C_W1 + (h * 2 + k + 1) * 16] = \
                w1[h, k * P:(k + 1) * P, :]
    a_src0 = np.asarray(inputs["a_src0"], f32)[..., 0]  # [8, 32]
    a_dst0 = np.asarray(inputs["a_dst0"], f32)[..., 0]
    a_src1 = np.asarray(inputs["a_src1"], f32)[..., 0]  # [8, 16]
    a_dst1 = np.asarray(inputs["a_dst1"], f32)[..., 0]
    for h in range(H):
        wpr[0:32, C_AS0 + h] = a_src0[h]
        wpr[0:16, C_AS1 + h] = a_src1[h]
        wpr[0:32, C_AD0 + 2 * h] = 0.2 * a_dst0[h]
        wpr[0:32, C_AD0 + 2 * h + 1] = 0.8 * a_dst0[h]
        wpr[0:16, C_AD1 + 2 * h] = 0.2 * a_dst1[h]
        wpr[0:16, C_AD1 + 2 * h + 1] = 0.8 * a_dst1[h]

    wpbf = np.zeros((P, BCOLS), f32)
    wpbf[:, CB_ID:CB_ID + P] = np.eye(P, dtype=f32)
    wpbf[0, CB_OR:CB_OR + P] = 1.0
    wpbf[:, CB_ONE] = 1.0
    wpbf[:, CB_EIGHT] = 8.0
    wpbf[:, CB_B1R:CB_B1R + 16] = np.asarray(inputs["b1"], f32).reshape(1, 16)
    wpbf[0, CB_O17:CB_O17 + 17] = 1.0
    for h in range(H):
        for k in range(2):
            wpbf[:, CB_W1 + (h * 2 + k) * 16: CB_W1 + (h * 2 + k + 1) * 16] = \
                w1[h, k * P:(k + 1) * P, :]
    wpb = wpbf.astype(ml_dtypes.bfloat16)
    return wpack, wpr, wpb


def _prep_inputs(inputs):
    import ml_dtypes
    x = np.asarray(inputs["x"], np.float32)
    verts = np.asarray(inputs["vertices"]).astype(np.int32)
    adj = np.asarray(inputs["adj"])
    emb_w = np.ascontiguousarray(np.asarray(inputs["emb_w"], np.float32))
    wpack, wpr, wpb = _make_wpack(inputs)
    wpack = np.ascontiguousarray(wpack)
    wpr = np.ascontiguousarray(wpr)
    wpb = np.ascontiguousarray(wpb)
    in_maps = []
    for c in range(BS):
        in_maps.append({
            "xt": np.ascontiguousarray(x[c].T),
            "verts": np.ascontiguousarray(verts[c].reshape(NCH, P).T),
            "adjb": np.ascontiguousarray(adj[c].astype(ml_dtypes.bfloat16)),
            "emb_w": emb_w,
            "wpack": wpack,
            "wpackr": wpr,
            "wpackb": wpb,
        })
    zero_b0 = bool(np.all(np.asarray(inputs["b0"]) == 0))
    zero_b1 = bool(np.all(np.asarray(inputs["b1"]) == 0))
    return in_maps, zero_b0, zero_b1


def _run(inputs, trace=False):
    from concourse.bass_utils import run_bass_kernel_spmd

    in_maps, zero_b0, zero_b1 = _prep_inputs(inputs)
    key = ("prog", zero_b0, zero_b1)
    if key not in _CACHE:
        _CACHE[key] = _build(zero_b0, zero_b1)
    nc = _CACHE[key]
    res = run_bass_kernel_spmd(
        nc, in_maps, list(range(BS)), trace=trace
    )
    out = np.stack([res.results[c]["out"] for c in range(BS)], axis=0)
    return out.astype(np.float32), res


def kernel(**inputs):
    out, _ = _run(inputs, trace=False)
    return out
